# revision 6
# baseline (speedup 1.0000x reference)
"""W8A16 column-parallel linear for TRN2, 8 NeuronCores.

Computes y = x @ (qweight * w_scales).T + bias with
  x        [8, 1, 8192]  fp16
  qweight  [28672, 8192] int8 (per-row symmetric quant)
  w_scales [28672, 1]    fp16
  bias     [28672]       fp16
  y        [8, 1, 28672] fp16

Sharding: column-parallel — each of the 8 cores owns 3584 output rows
(qweight/w_scales/bias shard), x replicated. No collectives; outputs are
concatenated on the host.

Per-core kernel: stream the int8 weight shard (transposed to [K, Nshard]
on host) from HBM in U-ktile groups, convert int8->fp16 on-chip (split
between VectorE and ScalarE to sustain HBM rate), then accumulate
out[8, 512]-chunk PSUM tiles with fp16 matmuls (stationary x^T tile,
moving weight tile). Bias is folded in as a K=1 matmul of ones^T @ bias
that opens each PSUM accumulation group; per-row scales are applied by a
single tensor_tensor multiply per chunk at the end.
"""

import numpy as np

import concourse.bass as bass
import concourse.bacc as bacc
import concourse.mybir as mybir
import concourse.tile as tile
from concourse.bass_utils import run_bass_kernel_spmd

B, S, K, N = 8, 1, 8192, 28672
M = B * S                 # 8 rows in the GEMM
NCORES = 8
NS = N // NCORES          # 3584 output rows per core
KT = K // 128             # 64 k-tiles
U = 4                     # k-tiles per DMA/conversion group
NG = KT // U              # 16 groups
NCHUNK = NS // 512        # 7 psum chunks of 512
DVE_N = 2240              # free-dim split of the int8->fp16 conversion
ACT_N = 3584 - DVE_N      # VectorE takes DVE_N, ScalarE takes ACT_N

_CACHE = {}

# chunk -> PE column-group (0,1,2 -> array cols 0-31/32-63/64-95). Three
# concurrent moving streams triple the PE's weight-streaming rate.
CHUNK_GRP = [0, 0, 0, 1, 1, 2, 2]
GRP_BASE = [32 * j for j in CHUNK_GRP]         # PSUM base partition per chunk
GRP_OFF = [0, 512, 1024, 1536, 2048, 2560, 3072]
GRP_SPAN = {0: (0, 1536), 1: (1536, 2560), 2: (2560, 3584)}


def _build():
    nc = bacc.Bacc()
    xp = nc.declare_dram_parameter("x", [128, KT * M], mybir.dt.float16, isOutput=False)
    qp = nc.declare_dram_parameter("qt", [K, NS], mybir.dt.int8, isOutput=False)
    sp = nc.declare_dram_parameter("s", [72, NS], mybir.dt.float16, isOutput=False)
    bp = nc.declare_dram_parameter("b", [1, NS], mybir.dt.float16, isOutput=False)
    op = nc.declare_dram_parameter("out", [M, NS], mybir.dt.float16, isOutput=True)

    qk = qp.rearrange("(kt p) n -> kt p n", p=128)  # [KT, 128, NS]

    # ramp-up: small leading groups so the first cast starts after ~0.5MB
    # of weight DMA instead of ~1.8MB
    GROUPS = [1, 1, 2] + [U] * ((KT - 4) // U)
    assert sum(GROUPS) == KT

    # per-ktile matmul issue order rotates through the PE column groups so
    # the three streams start back-to-back instead of blocking each other
    ISSUE = [0, 3, 5, 1, 4, 6, 2]

    with tile.TileContext(nc) as tc:
        with (
            tc.tile_pool(name="const", bufs=1) as constp,
            tc.tile_pool(name="wq", bufs=3) as wqp,
            tc.tile_pool(name="wf", bufs=3) as wfp,
            tc.tile_pool(name="psum", bufs=1, space="PSUM") as psp,
            tc.tile_pool(name="outp", bufs=1) as outp,
        ):
            xsb = constp.tile([128, KT * M], mybir.dt.float16, tag="xsb")
            sb = constp.tile([72, NS], mybir.dt.float16, tag="sb")
            b1 = constp.tile([1, NS], mybir.dt.float16, tag="b1")
            ones = constp.tile([1, M], mybir.dt.float16, tag="ones")

            # first weight group goes down the HWDGE queue ahead of
            # everything; constants ride the SWDGE (gpsimd) path so they
            # don't delay the weight stream
            wq0 = wqp.tile([128, GROUPS[0], NS], mybir.dt.int8, tag="wq")
            nc.sync.dma_start(wq0[:], qk[0:GROUPS[0]].rearrange("u p n -> p u n"))
            nc.gpsimd.dma_start(xsb[:], xp[:])
            nc.gpsimd.dma_start(sb[:], sp[:])
            nc.gpsimd.dma_start(b1[:], bp[:])
            nc.gpsimd.memset(ones[:], 1.0)

            psums = []
            for c in range(NCHUNK):
                pt = psp.tile([128, 512], mybir.dt.float32, tag=f"ps{c}")
                psums.append(pt)
            for c in ISSUE:
                lo = GRP_BASE[c]
                # bias row opens the accumulation group: psum = ones^T @ bias
                nc.tensor.matmul(
                    psums[c][lo:lo + M, :], ones[:], b1[:, c * 512:(c + 1) * 512],
                    start=True, stop=False,
                )

            kt0 = 0
            for g, gu in enumerate(GROUPS):
                if g == 0:
                    wq = wq0
                else:
                    wq = wqp.tile([128, gu, NS], mybir.dt.int8, tag="wq")
                    nc.sync.dma_start(
                        wq[:], qk[kt0:kt0 + gu].rearrange("u p n -> p u n")
                    )
                wf = wfp.tile([128, gu, NS], mybir.dt.float16, tag="wf")
                nc.vector.tensor_copy(wf[:, :, 0:DVE_N], wq[:, :, 0:DVE_N])
                nc.scalar.activation(
                    wf[:, :, DVE_N:NS], wq[:, :, DVE_N:NS],
                    mybir.ActivationFunctionType.Copy,
                )
                for u in range(gu):
                    kt = kt0 + u
                    last = kt == KT - 1
                    for c in ISSUE:
                        lo = GRP_BASE[c]
                        nc.tensor.matmul(
                            psums[c][lo:lo + M, :],
                            xsb[:, kt * M:(kt + 1) * M],
                            wf[:, u, c * 512:(c + 1) * 512],
                            start=False, stop=last,
                        )
                kt0 += gu

            osb = outp.tile([72, NS], mybir.dt.float16, tag="osb")
            for c in range(NCHUNK):
                lo = GRP_BASE[c]
                nc.vector.tensor_mul(
                    osb[lo:lo + M, c * 512:(c + 1) * 512],
                    psums[c][lo:lo + M, :],
                    sb[lo:lo + M, c * 512:(c + 1) * 512],
                )
            for j, (nlo, nhi) in GRP_SPAN.items():
                plo = 32 * j
                nc.sync.dma_start(op[:, nlo:nhi], osb[plo:plo + M, nlo:nhi])

    nc.compile()
    return nc


def _get_nc():
    if "nc" not in _CACHE:
        _CACHE["nc"] = _build()
    return _CACHE["nc"]


def _prep_inputs(x, qweight, w_scales, bias):
    x2 = np.asarray(x, dtype=np.float16).reshape(M, K)
    # xsb[p, kt*M + m] = x[m, kt*128 + p]
    xsb = np.ascontiguousarray(
        x2.T.reshape(KT, 128, M).transpose(1, 0, 2).reshape(128, KT * M)
    )
    qweight = np.asarray(qweight)
    w_scales = np.asarray(w_scales, dtype=np.float16).reshape(N)
    bias = np.asarray(bias, dtype=np.float16).reshape(N)
    in_maps = []
    for c in range(NCORES):
        sl = slice(c * NS, (c + 1) * NS)
        qt = np.ascontiguousarray(qweight[sl, :].T)          # [K, NS] int8
        sb = np.zeros((72, NS), dtype=np.float16)
        for j in range(3):
            sb[32 * j:32 * j + M, :] = w_scales[sl]           # [72, NS] fp16
        b1 = np.ascontiguousarray(bias[sl].reshape(1, NS))    # [1, NS] fp16
        in_maps.append({"x": xsb, "qt": qt, "s": sb, "b": b1})
    return in_maps


def _run(x, qweight, w_scales, bias, trace=False):
    nc = _get_nc()
    in_maps = _prep_inputs(x, qweight, w_scales, bias)
    res = run_bass_kernel_spmd(
        nc, in_maps, core_ids=list(range(NCORES)), trace=trace
    )
    y = np.concatenate(
        [np.asarray(res.results[c]["out"]) for c in range(NCORES)], axis=1
    )
    return y.reshape(B, S, N).astype(np.float16), res


def kernel(x, qweight, w_scales, bias):
    y, _ = _run(x, qweight, w_scales, bias, trace=False)
    return y


def kernel_traced(x, qweight, w_scales, bias):
    """Like kernel() but also returns the BassKernelResults (exec_time_ns)."""
    return _run(x, qweight, w_scales, bias, trace=True)
